# revision 33
# baseline (speedup 1.0000x reference)
"""Chamfer-style loss kernel for Trainium2 (8 NeuronCores, SPMD).

Problem: y_pred [8192,2], y_true [8192,2] (fp32).
  d[n,m] = ||p_n - t_m||;  loss = (sum_n min_m d + sum_m min_n d) / 8192

Strategy per core k (shard y_pred rows, 1024 per core):
  - Augmented K=4 matmul on PE computes the full squared-distance tile
    S[n,m] = |p_n|^2 + |t_m|^2 - 2 p.t  directly in PSUM:
       lhsT = [-2px; -2py; 1; |p|^2]  (4 x 1024)
       rhs  = [tx; ty; |t|^2; 1]      (4 x 8192)
    K=4 uses only 4 PE rows, so 4 matmuls are packed onto row quadrants
    via tile_position (lhs/rhs pre-replicated at partition offsets
    0/32/64/96 on the host).
  - ACT copies PSUM->SBUF as bf16 (min chains run at DVE 2x mode in bf16).
  - DVE tensor_tensor(min) chains process chunk PAIRS (1024 columns of
    y_true at a time): row-min accumulation plus a col-min tree; the
    partition axis is folded 128->32-groups with the DVE 32x32 stream
    transpose, then a tiny strided reduce. Cross-quadrant fold happens
    once at the end via 2 DMA xbar transposes.
  - sqrt commutes with min, so sqrt only on the 16K final mins.
  - One AllReduce(min) over [129,64] fp32: rows 0..127 = col-min partials
    (all 8192 m), row 128 = one-hot gather of per-core row-min sqrt-sums.
  - Every core then computes the identical final scalar; host takes core 0.
"""

import sys

if "/opt/trn_rl_repo" not in sys.path:
    sys.path.insert(0, "/opt/trn_rl_repo")

import numpy as np

import concourse.bass as bass
import concourse.bacc as bacc
import concourse.tile as tile
from concourse import mybir
from concourse.bass_utils import run_bass_kernel_spmd
from concourse.tile_rust import add_dep_helper

F32 = mybir.dt.float32
BF16 = mybir.dt.bfloat16
MIN = mybir.AluOpType.min
ADD = mybir.AluOpType.add
X = mybir.AxisListType.X

N_CORES = 8
N = 8192          # y_pred points
M = 8192          # y_true points
N_LOC = N // N_CORES   # 1024 rows per core
N_BLK = N_LOC // 128   # 8 row blocks of 128 partitions
N_PAIR = 8             # pairs of 512-column chunks (1024 cols each)
CHUNK = 512
BIG = 3.0e38

TRACE = False          # set True by test harness to capture HW profile
LAST_RESULTS = None    # BassKernelResults of the most recent run

_CACHE = {}


def _build_program():
    nc = bacc.Bacc(
        "TRN2",
        target_bir_lowering=False,
        debug=False,
        num_devices=N_CORES,
    )

    lhs_d = nc.dram_tensor("lhs", [16, N_LOC], F32, kind="ExternalInput")
    rhs_d = nc.dram_tensor("rhs", [16, M], F32, kind="ExternalInput")
    sel_d = nc.dram_tensor("sel", [1, 64], F32, kind="ExternalInput")
    selbig_d = nc.dram_tensor("selbig", [1, 64], F32, kind="ExternalInput")
    out_d = nc.dram_tensor("out", [1, 1], F32, kind="ExternalOutput")

    with tile.TileContext(nc) as tc:
        with (
            tc.tile_pool(name="const", bufs=1) as const_pool,
            tc.tile_pool(name="acc", bufs=1) as acc_pool,
            tc.tile_pool(name="chunk", bufs=3) as chunk_pool,
            tc.tile_pool(name="tree", bufs=2) as tree_pool,
            tc.tile_pool(name="fin", bufs=1) as fin_pool,
            tc.tile_pool(name="mm", bufs=2, space="PSUM") as mm_pool,
            tc.tile_pool(name="dram", bufs=1, space="DRAM") as dram_pool,
        ):
            # ---- constants / inputs to SBUF ----
            # lhs/rhs live at partition offsets 0/32/64/96 so four K=4
            # matmuls can run concurrently on the four PE row quadrants.
            lhs_sb = const_pool.tile([128, N_LOC], F32, padded_shape=[128, N_LOC])
            rhs_sb = const_pool.tile([128, M], F32, padded_shape=[128, M])
            sel_sb = const_pool.tile([1, 64], F32)
            selbig_sb = const_pool.tile([1, 64], F32)
            ones_sb = const_pool.tile([128, 1], F32)

            # lhs first (the weights gate the very first LDWEIGHTS), then
            # rhs in column pieces, earliest columns first, so the first
            # matmuls unblock as soon as their slice lands
            engs = [nc.sync, nc.scalar]
            for r in range(4):
                engs[r % 2].dma_start(
                    lhs_sb[32 * r:32 * r + 4, :], lhs_d.ap()[4 * r:4 * r + 4, :]
                )
            for piece in range(2):
                lo, hi = piece * 4096, (piece + 1) * 4096
                for r in range(4):
                    engs[(piece * 4 + r) % 2].dma_start(
                        rhs_sb[32 * r:32 * r + 4, lo:hi],
                        rhs_d.ap()[4 * r:4 * r + 4, lo:hi],
                    )
            nc.sync.dma_start(sel_sb[:, :], sel_d.ap())
            nc.scalar.dma_start(selbig_sb[:, :], selbig_d.ap())
            nc.vector.memset(ones_sb[:, :], 1.0)

            # ---- persistent accumulators ----
            # row-min candidates, ping-pong buffers: [128, 8 (row-block), 512]
            rowacc_a = acc_pool.tile([128, N_BLK * CHUNK], BF16)
            rowacc_b = acc_pool.tile([128, N_BLK * CHUNK], BF16)
            # quadrant-folded col-min: [32P+q, j] = min over partitions
            # 32P..32P+31 of column m(j)
            colcand32 = acc_pool.tile([128, 256], BF16)
            rowaccs = [rowacc_a, rowacc_b]

            # ---- main loop over pairs of 512-column chunks ----
            for p in range(N_PAIR):
                pair_sb = chunk_pool.tile(
                    [128, 2 * N_BLK * CHUNK], BF16, name="pair_sb", tag="chunk"
                )
                # PE: S tiles for both chunks x 8 row-blocks, packed
                # 4-at-a-time onto the PE row quadrants.
                for h in range(4):      # (chunk, half) = (2p + h//2, h%2)
                    c = 2 * p + h // 2
                    g = h % 2
                    mm_ps = mm_pool.tile(
                        [128, 4 * CHUNK], F32, name="mm_ps", tag="mm"
                    )
                    for r in range(4):
                        i = 4 * g + r
                        nc.tensor.matmul(
                            mm_ps[:, r * CHUNK:(r + 1) * CHUNK],
                            lhs_sb[32 * r:32 * r + 4, i * 128:(i + 1) * 128],
                            rhs_sb[32 * r:32 * r + 4, c * CHUNK:(c + 1) * CHUNK],
                            start=True,
                            stop=True,
                            tile_position=(32 * r, 0),
                        )
                    # ACT: PSUM fp32 -> SBUF bf16
                    nc.scalar.copy(
                        pair_sb[:, h * 2048:(h + 1) * 2048], mm_ps[:, :]
                    )

                # DVE row chain: min across the two chunks, then into rowacc
                if p == 0:
                    # split so the first half starts after 3 ACT copies
                    nc.vector.tensor_tensor(
                        rowaccs[1][:, 0:2048],
                        pair_sb[:, 0:2048],
                        pair_sb[:, 4096:6144],
                        MIN,
                    )
                    nc.vector.tensor_tensor(
                        rowaccs[1][:, 2048:4096],
                        pair_sb[:, 2048:4096],
                        pair_sb[:, 6144:8192],
                        MIN,
                    )
                else:
                    trow = tree_pool.tile(
                        [128, N_BLK * CHUNK], BF16, name="trow", tag="trow"
                    )
                    nc.vector.tensor_tensor(
                        trow[:, :], pair_sb[:, 0:4096], pair_sb[:, 4096:8192], MIN
                    )
                    src = rowaccs[p % 2]
                    dst = rowaccs[(p + 1) % 2]
                    nc.vector.tensor_tensor(dst[:, :], src[:, :], trow[:, :], MIN)

                # DVE col tree over the 8 row-blocks (both chunks at once)
                pv = pair_sb.rearrange("q (c i f) -> q c i f", c=2, i=N_BLK)
                t1 = tree_pool.tile([128, 4096], BF16, name="t1", tag="t1")
                t2 = tree_pool.tile([128, 2048], BF16, name="t2", tag="t2")
                t3 = tree_pool.tile([128, 1024], BF16, name="t3", tag="t3")
                t1v = t1.rearrange("q (c i f) -> q c i f", c=2, i=4)
                t2v = t2.rearrange("q (c i f) -> q c i f", c=2, i=2)
                nc.vector.tensor_tensor(
                    t1v[:, :, :, :], pv[:, :, 0:4, :], pv[:, :, 4:8, :], MIN
                )
                nc.vector.tensor_tensor(
                    t2v[:, :, :, :], t1v[:, :, 0:2, :], t1v[:, :, 2:4, :], MIN
                )
                nc.vector.tensor_tensor(
                    t3.rearrange("q (c f) -> q c f", c=2),
                    t2v[:, :, 0, :],
                    t2v[:, :, 1, :],
                    MIN,
                )
                # DVE 32x32 block transpose + within-quadrant reduce:
                # vt[32P+q, 32F+g] = t3[32P+g, 32F+q]
                vt = tree_pool.tile([128, 1024], BF16, name="vt", tag="vt")
                nc.vector.transpose(vt[:, :], t3[:, :])
                nc.vector.tensor_reduce(
                    colcand32[:, p * 32:(p + 1) * 32],
                    vt.rearrange("q (F g) -> q F g", g=32),
                    axis=X,
                    op=MIN,
                )

                if p == 4:
                    # first half of colcand32 (pairs 0-3) is complete: fold
                    # quadrants and launch AllReduce #1 now so it overlaps
                    # the rest of the loop on the collective hardware
                    # (emitted one pair late so the DVE never waits on the
                    # fold transpose)
                    tb0 = fin_pool.tile([128, 128], BF16)
                    nc.sync.dma_start_transpose(tb0[:, :], colcand32[:, 0:128])
                    colA = fin_pool.tile([128, 32], BF16)
                    nc.vector.tensor_reduce(
                        colA[:, :],
                        tb0.rearrange("j (P q) -> j q P", P=4),
                        axis=X,
                        op=MIN,
                    )
                    ar1_in = dram_pool.tile([128, 32], BF16)
                    ar1_out = dram_pool.tile([128, 32], BF16, addr_space="Shared")
                    nc.sync.dma_start(ar1_in[:, :], colA[:, :])
                    nc.gpsimd.collective_compute(
                        "AllReduce",
                        MIN,
                        replica_groups=[list(range(N_CORES))],
                        ins=[ar1_in[:, :].opt()],
                        outs=[ar1_out[:, :].opt()],
                    )

            # second-half fold transpose kicked off first (sync queue) so it
            # overlaps the row finalization below on the DVE
            tb1 = fin_pool.tile([128, 128], BF16)
            nc.sync.dma_start_transpose(tb1[:, :], colcand32[:, 128:256])

            # ---- local row-min finalization (small TT tree + reduce) ----
            rowacc = rowaccs[N_PAIR % 2]
            rview = rowacc.rearrange("q (i f) -> q i f", i=N_BLK)
            rt1 = fin_pool.tile([128, N_BLK * 256], BF16)
            rt2 = fin_pool.tile([128, N_BLK * 128], BF16)
            r1v = rt1.rearrange("q (i f) -> q i f", i=N_BLK)
            r2v = rt2.rearrange("q (i f) -> q i f", i=N_BLK)
            nc.vector.tensor_tensor(
                r1v[:, :, :], rview[:, :, 0:256], rview[:, :, 256:512], MIN
            )
            nc.vector.tensor_tensor(
                r2v[:, :, :], r1v[:, :, 0:128], r1v[:, :, 128:256], MIN
            )
            rowmin8 = fin_pool.tile([128, N_BLK], F32)
            nc.vector.tensor_reduce(rowmin8[:, :], r2v, axis=X, op=MIN)

            # ---- cross-quadrant fold of the second colcand32 half ----
            colB = fin_pool.tile([128, 32], F32)
            nc.vector.tensor_reduce(
                colB[:, :],
                tb1.rearrange("j (P q) -> j q P", P=4),
                axis=X,
                op=MIN,
            )
            # clamp negatives (cancellation noise), sqrt, sum over free dim
            nc.vector.tensor_scalar_max(rowmin8[:, :], rowmin8[:, :], 0.0)
            rowd = fin_pool.tile([128, N_BLK], F32)
            rowpart = fin_pool.tile([128, 1], F32)
            i_rowsqrt = nc.scalar.activation(
                rowd[:, :], rowmin8[:, :],
                mybir.ActivationFunctionType.Sqrt,
                accum_out=rowpart[:, :],
            )
            # partition sum -> scalar
            ps_row = mm_pool.tile([128, 4 * CHUNK], F32, name="ps_row", tag="mm")
            nc.tensor.matmul(
                ps_row[0:1, 0:1], ones_sb[:, :], rowpart[:, :],
                start=True, stop=True,
            )
            rowsum_sb = fin_pool.tile([1, 1], F32)
            nc.scalar.copy(rowsum_sb[:, :], ps_row[0:1, 0:1])

            # ---- AllGather #2 over [128, 33] fp32: cols 0..31 = colB,
            # col 32 row 0 = this core's rowsum. AllGather is ~2x cheaper
            # than AllReduce; the 8-way min is done locally afterwards.
            ar2_in = dram_pool.tile([128, 33], F32)
            ag2_out = dram_pool.tile([1024, 33], F32, addr_space="Shared")
            nc.sync.dma_start(ar2_in[0:128, 0:32], colB[:, :])
            i_gdma = nc.sync.dma_start(ar2_in[0:1, 32:33], rowsum_sb[:, :])
            nc.gpsimd.collective_compute(
                "AllGather",
                mybir.AluOpType.bypass,
                replica_groups=[list(range(N_CORES))],
                ins=[ar2_in[:, :].opt()],
                outs=[ag2_out[:, :].opt()],
            )

            # ---- global finalization (identical on every core) ----
            cminA = fin_pool.tile([128, 32], BF16)
            call = fin_pool.tile([128, 8 * 33], F32)
            # pin AR#1 consumers behind the tail of the main pipeline so
            # the scheduler can't park them at the head of an engine queue
            # (which would stall the whole loop on the collective)
            i_cA = nc.sync.dma_start(cminA[:, :], ar1_out[:, :])
            add_dep_helper(i_cA.ins, i_gdma.ins, sync=False,
                           reason="AR1 consumer after loop tail")
            # one fused load: dram block-major -> sbuf partition-major
            nc.sync.dma_start(
                call[:, :],
                ag2_out.rearrange("(j r) c -> r j c", j=N_CORES),
            )

            i_clA = nc.vector.tensor_scalar_max(cminA[:, :], cminA[:, :], 0.0)
            add_dep_helper(i_clA.ins, i_gdma.ins, sync=False,
                           reason="AR1 consumer after loop tail")
            # local 8-way min over the gathered colB blocks
            cv = call.rearrange("r (j q) -> r j q", j=N_CORES)
            m1 = fin_pool.tile([128, 4 * 32], F32)
            m2 = fin_pool.tile([128, 2 * 32], F32)
            cminB = fin_pool.tile([128, 32], F32)
            m1v = m1.rearrange("r (j q) -> r j q", j=4)
            m2v = m2.rearrange("r (j q) -> r j q", j=2)
            nc.vector.tensor_tensor(
                m1v[:, :, :], cv[:, 0:4, 0:32], cv[:, 4:8, 0:32], MIN
            )
            nc.vector.tensor_tensor(
                m2v[:, :, :], m1v[:, 0:2, :], m1v[:, 2:4, :], MIN
            )
            nc.vector.tensor_tensor(
                cminB.rearrange("r (j q) -> r j q", j=1),
                m2v[:, 0:1, :],
                m2v[:, 1:2, :],
                MIN,
            )
            nc.vector.tensor_scalar_max(cminB[:, :], cminB[:, :], 0.0)
            cdA = fin_pool.tile([128, 32], F32)
            cdB = fin_pool.tile([128, 32], F32)
            colpartA = fin_pool.tile([128, 1], F32)
            colpartB = fin_pool.tile([128, 1], F32)
            i_sqA = nc.scalar.activation(
                cdA[:, :], cminA[:, :],
                mybir.ActivationFunctionType.Sqrt,
                accum_out=colpartA[:, :],
            )
            add_dep_helper(i_sqA.ins, i_rowsqrt.ins, sync=False,
                           reason="AR1 consumer after loop tail")
            nc.scalar.activation(
                cdB[:, :], cminB[:, :],
                mybir.ActivationFunctionType.Sqrt,
                accum_out=colpartB[:, :],
            )
            colpart = fin_pool.tile([128, 1], F32)
            nc.vector.tensor_tensor(
                colpart[:, :], colpartA[:, :], colpartB[:, :], ADD
            )
            ps_col = mm_pool.tile([128, 4 * CHUNK], F32, name="ps_col", tag="mm")
            nc.tensor.matmul(
                ps_col[0:1, 0:1], ones_sb[:, :], colpart[:, :],
                start=True, stop=True,
            )
            colsum_sb = fin_pool.tile([1, 1], F32)
            nc.scalar.copy(colsum_sb[:, :], ps_col[0:1, 0:1])

            # rowsums: block j's col 32, row 0 -> strided [1, 8] view
            rtot = fin_pool.tile([1, 1], F32)
            nc.vector.tensor_reduce(
                rtot[:, :],
                call.rearrange("r (j q) -> r j q", j=N_CORES)[0:1, :, 32],
                axis=X,
                op=ADD,
            )

            fin = fin_pool.tile([1, 1], F32)
            nc.vector.tensor_tensor(fin[:, :], colsum_sb[:, :], rtot[:, :], ADD)
            out_sb = fin_pool.tile([1, 1], F32)
            nc.scalar.mul(out_sb[:, :], fin[:, :], 1.0 / M)
            nc.sync.dma_start(out_d.ap(), out_sb[:, :])

    nc.compile()
    return nc


def _prep_inputs(y_pred, y_true):
    p = np.ascontiguousarray(np.asarray(y_pred, dtype=np.float32).reshape(-1, 2))
    t = np.ascontiguousarray(np.asarray(y_true, dtype=np.float32).reshape(-1, 2))
    assert p.shape == (N, 2) and t.shape == (M, 2)

    rhs4 = np.empty((4, M), dtype=np.float32)
    rhs4[0] = t[:, 0]
    rhs4[1] = t[:, 1]
    rhs4[2] = t[:, 0] * t[:, 0] + t[:, 1] * t[:, 1]
    rhs4[3] = 1.0
    rhs = np.tile(rhs4, (4, 1))

    in_maps = []
    for k in range(N_CORES):
        pk = p[k * N_LOC:(k + 1) * N_LOC]
        lhs4 = np.empty((4, N_LOC), dtype=np.float32)
        lhs4[0] = -2.0 * pk[:, 0]
        lhs4[1] = -2.0 * pk[:, 1]
        lhs4[2] = 1.0
        lhs4[3] = pk[:, 0] * pk[:, 0] + pk[:, 1] * pk[:, 1]
        lhs = np.tile(lhs4, (4, 1))
        sel = np.zeros((1, 64), dtype=np.float32)
        sel[0, k] = 1.0
        selbig = np.full((1, 64), BIG, dtype=np.float32)
        selbig[0, k] = 0.0
        in_maps.append({"lhs": lhs, "rhs": rhs, "sel": sel, "selbig": selbig})
    return in_maps


def kernel(y_pred, y_true):
    global LAST_RESULTS
    if "nc" not in _CACHE:
        _CACHE["nc"] = _build_program()
    nc = _CACHE["nc"]
    in_maps = _prep_inputs(y_pred, y_true)
    res = run_bass_kernel_spmd(
        nc,
        in_maps,
        core_ids=list(range(N_CORES)),
        trace=TRACE,
    )
    LAST_RESULTS = res
    return np.asarray(res.results[0]["out"], dtype=np.float32).reshape(())[()]


# revision 34
# speedup vs baseline: 1.1387x; 1.1387x over previous
"""Chamfer-style loss kernel for Trainium2 (8 NeuronCores, SPMD).

Problem: y_pred [8192,2], y_true [8192,2] (fp32).
  d[n,m] = ||p_n - t_m||;  loss = (sum_n min_m d + sum_m min_n d) / 8192

Strategy per core k (shard y_pred rows, 1024 per core):
  - Augmented K=4 matmul on PE computes the full squared-distance tile
    S[n,m] = |p_n|^2 + |t_m|^2 - 2 p.t  directly in PSUM:
       lhsT = [-2px; -2py; 1; |p|^2]  (4 x 1024)
       rhs  = [tx; ty; |t|^2; 1]      (4 x 8192)
    K=4 uses only 4 PE rows, so 4 matmuls are packed onto row quadrants
    via tile_position (lhs/rhs pre-replicated at partition offsets
    0/32/64/96 on the host).
  - ACT copies PSUM->SBUF as bf16 (min chains run at DVE 2x mode in bf16).
  - DVE tensor_tensor(min) chains process chunk PAIRS (1024 columns of
    y_true at a time): row-min accumulation plus a col-min tree; the
    partition axis is folded 128->32-groups with the DVE 32x32 stream
    transpose, then a tiny strided reduce. Cross-quadrant fold happens
    once at the end via 2 DMA xbar transposes.
  - sqrt commutes with min, so sqrt only on the 16K final mins.
  - One AllReduce(min) over [129,64] fp32: rows 0..127 = col-min partials
    (all 8192 m), row 128 = one-hot gather of per-core row-min sqrt-sums.
  - Every core then computes the identical final scalar; host takes core 0.
"""

import sys

if "/opt/trn_rl_repo" not in sys.path:
    sys.path.insert(0, "/opt/trn_rl_repo")

import numpy as np

import concourse.bass as bass
import concourse.bacc as bacc
import concourse.tile as tile
from concourse import mybir
from concourse.bass_utils import run_bass_kernel_spmd
from concourse.tile_rust import add_dep_helper

F32 = mybir.dt.float32
BF16 = mybir.dt.bfloat16
MIN = mybir.AluOpType.min
ADD = mybir.AluOpType.add
X = mybir.AxisListType.X

N_CORES = 8
N = 8192          # y_pred points
M = 8192          # y_true points
N_LOC = N // N_CORES   # 1024 rows per core
N_BLK = N_LOC // 128   # 8 row blocks of 128 partitions
N_PAIR = 8             # pairs of 512-column chunks (1024 cols each)
CHUNK = 512
BIG = 3.0e38

TRACE = False          # set True by test harness to capture HW profile
LAST_RESULTS = None    # BassKernelResults of the most recent run

_CACHE = {}


def _build_program():
    nc = bacc.Bacc(
        "TRN2",
        target_bir_lowering=False,
        debug=False,
        num_devices=N_CORES,
    )

    lhs_d = nc.dram_tensor("lhs", [16, N_LOC], F32, kind="ExternalInput")
    rhs_d = nc.dram_tensor("rhs", [16, M], F32, kind="ExternalInput")
    sel_d = nc.dram_tensor("sel", [1, 64], F32, kind="ExternalInput")
    selbig_d = nc.dram_tensor("selbig", [1, 64], F32, kind="ExternalInput")
    out_d = nc.dram_tensor("out", [1, 1], F32, kind="ExternalOutput")

    with tile.TileContext(nc) as tc:
        with (
            tc.tile_pool(name="const", bufs=1) as const_pool,
            tc.tile_pool(name="acc", bufs=1) as acc_pool,
            tc.tile_pool(name="chunk", bufs=3) as chunk_pool,
            tc.tile_pool(name="tree", bufs=2) as tree_pool,
            tc.tile_pool(name="fin", bufs=1) as fin_pool,
            tc.tile_pool(name="mm", bufs=2, space="PSUM") as mm_pool,
            tc.tile_pool(name="dram", bufs=1, space="DRAM") as dram_pool,
        ):
            # ---- constants / inputs to SBUF ----
            # lhs/rhs live at partition offsets 0/32/64/96 so four K=4
            # matmuls can run concurrently on the four PE row quadrants.
            lhs_sb = const_pool.tile([128, N_LOC], F32, padded_shape=[128, N_LOC])
            rhs_sb = const_pool.tile([128, M], F32, padded_shape=[128, M])
            sel_sb = const_pool.tile([1, 64], F32)
            selbig_sb = const_pool.tile([1, 64], F32)
            ones_sb = const_pool.tile([128, 1], F32)

            # lhs first (the weights gate the very first LDWEIGHTS), then
            # rhs in column pieces, earliest columns first, so the first
            # matmuls unblock as soon as their slice lands
            engs = [nc.sync, nc.scalar]
            for r in range(4):
                engs[r % 2].dma_start(
                    lhs_sb[32 * r:32 * r + 4, :], lhs_d.ap()[4 * r:4 * r + 4, :]
                )
            for piece in range(2):
                lo, hi = piece * 4096, (piece + 1) * 4096
                for r in range(4):
                    engs[(piece * 4 + r) % 2].dma_start(
                        rhs_sb[32 * r:32 * r + 4, lo:hi],
                        rhs_d.ap()[4 * r:4 * r + 4, lo:hi],
                    )
            nc.sync.dma_start(sel_sb[:, :], sel_d.ap())
            nc.scalar.dma_start(selbig_sb[:, :], selbig_d.ap())
            nc.vector.memset(ones_sb[:, :], 1.0)

            # ---- persistent accumulators ----
            # row-min candidates, ping-pong buffers: [128, 8 (row-block), 512]
            rowacc_a = acc_pool.tile([128, N_BLK * CHUNK], BF16)
            rowacc_b = acc_pool.tile([128, N_BLK * CHUNK], BF16)
            # quadrant-folded col-min: [32P+q, j] = min over partitions
            # 32P..32P+31 of column m(j)
            colcand32 = acc_pool.tile([128, 256], BF16)
            rowaccs = [rowacc_a, rowacc_b]

            # ---- main loop over pairs of 512-column chunks ----
            for p in range(N_PAIR):
                pair_sb = chunk_pool.tile(
                    [128, 2 * N_BLK * CHUNK], BF16, name="pair_sb", tag="chunk"
                )
                # PE: S tiles for both chunks x 8 row-blocks, packed
                # 4-at-a-time onto the PE row quadrants.
                for h in range(4):      # (chunk, half) = (2p + h//2, h%2)
                    c = 2 * p + h // 2
                    g = h % 2
                    mm_ps = mm_pool.tile(
                        [128, 4 * CHUNK], F32, name="mm_ps", tag="mm"
                    )
                    for r in range(4):
                        i = 4 * g + r
                        nc.tensor.matmul(
                            mm_ps[:, r * CHUNK:(r + 1) * CHUNK],
                            lhs_sb[32 * r:32 * r + 4, i * 128:(i + 1) * 128],
                            rhs_sb[32 * r:32 * r + 4, c * CHUNK:(c + 1) * CHUNK],
                            start=True,
                            stop=True,
                            tile_position=(32 * r, 0),
                        )
                    # ACT: PSUM fp32 -> SBUF bf16
                    nc.scalar.copy(
                        pair_sb[:, h * 2048:(h + 1) * 2048], mm_ps[:, :]
                    )

                # DVE row chain: min across the two chunks, then into rowacc
                if p == 0:
                    nc.vector.tensor_tensor(
                        rowaccs[1][:, :],
                        pair_sb[:, 0:4096],
                        pair_sb[:, 4096:8192],
                        MIN,
                    )
                else:
                    trow = tree_pool.tile(
                        [128, N_BLK * CHUNK], BF16, name="trow", tag="trow"
                    )
                    nc.vector.tensor_tensor(
                        trow[:, :], pair_sb[:, 0:4096], pair_sb[:, 4096:8192], MIN
                    )
                    src = rowaccs[p % 2]
                    dst = rowaccs[(p + 1) % 2]
                    nc.vector.tensor_tensor(dst[:, :], src[:, :], trow[:, :], MIN)

                # DVE col tree over the 8 row-blocks (both chunks at once)
                pv = pair_sb.rearrange("q (c i f) -> q c i f", c=2, i=N_BLK)
                t1 = tree_pool.tile([128, 4096], BF16, name="t1", tag="t1")
                t2 = tree_pool.tile([128, 2048], BF16, name="t2", tag="t2")
                t3 = tree_pool.tile([128, 1024], BF16, name="t3", tag="t3")
                t1v = t1.rearrange("q (c i f) -> q c i f", c=2, i=4)
                t2v = t2.rearrange("q (c i f) -> q c i f", c=2, i=2)
                nc.vector.tensor_tensor(
                    t1v[:, :, :, :], pv[:, :, 0:4, :], pv[:, :, 4:8, :], MIN
                )
                nc.vector.tensor_tensor(
                    t2v[:, :, :, :], t1v[:, :, 0:2, :], t1v[:, :, 2:4, :], MIN
                )
                nc.vector.tensor_tensor(
                    t3.rearrange("q (c f) -> q c f", c=2),
                    t2v[:, :, 0, :],
                    t2v[:, :, 1, :],
                    MIN,
                )
                # DVE 32x32 block transpose + within-quadrant reduce:
                # vt[32P+q, 32F+g] = t3[32P+g, 32F+q]
                vt = tree_pool.tile([128, 1024], BF16, name="vt", tag="vt")
                nc.vector.transpose(vt[:, :], t3[:, :])
                nc.vector.tensor_reduce(
                    colcand32[:, p * 32:(p + 1) * 32],
                    vt.rearrange("q (F g) -> q F g", g=32),
                    axis=X,
                    op=MIN,
                )

                if p == 4:
                    # first half of colcand32 (pairs 0-3) is complete: fold
                    # quadrants and launch AllReduce #1 now so it overlaps
                    # the rest of the loop on the collective hardware
                    # (emitted one pair late so the DVE never waits on the
                    # fold transpose)
                    tb0 = fin_pool.tile([128, 128], BF16)
                    nc.sync.dma_start_transpose(tb0[:, :], colcand32[:, 0:128])
                    colA = fin_pool.tile([128, 32], BF16)
                    nc.vector.tensor_reduce(
                        colA[:, :],
                        tb0.rearrange("j (P q) -> j q P", P=4),
                        axis=X,
                        op=MIN,
                    )
                    ar1_in = dram_pool.tile([128, 32], BF16)
                    ar1_out = dram_pool.tile([128, 32], BF16, addr_space="Shared")
                    nc.sync.dma_start(ar1_in[:, :], colA[:, :])
                    nc.gpsimd.collective_compute(
                        "AllReduce",
                        MIN,
                        replica_groups=[list(range(N_CORES))],
                        ins=[ar1_in[:, :].opt()],
                        outs=[ar1_out[:, :].opt()],
                    )

            # second-half fold transpose kicked off first (sync queue) so it
            # overlaps the row finalization below on the DVE
            tb1 = fin_pool.tile([128, 128], BF16)
            nc.sync.dma_start_transpose(tb1[:, :], colcand32[:, 128:256])

            # ---- local row-min finalization (small TT tree + reduce) ----
            rowacc = rowaccs[N_PAIR % 2]
            rview = rowacc.rearrange("q (i f) -> q i f", i=N_BLK)
            rt1 = fin_pool.tile([128, N_BLK * 256], BF16)
            rt2 = fin_pool.tile([128, N_BLK * 128], BF16)
            r1v = rt1.rearrange("q (i f) -> q i f", i=N_BLK)
            r2v = rt2.rearrange("q (i f) -> q i f", i=N_BLK)
            nc.vector.tensor_tensor(
                r1v[:, :, :], rview[:, :, 0:256], rview[:, :, 256:512], MIN
            )
            nc.vector.tensor_tensor(
                r2v[:, :, :], r1v[:, :, 0:128], r1v[:, :, 128:256], MIN
            )
            rowmin8 = fin_pool.tile([128, N_BLK], F32)
            nc.vector.tensor_reduce(rowmin8[:, :], r2v, axis=X, op=MIN)

            # ---- cross-quadrant fold of the second colcand32 half ----
            colB = fin_pool.tile([128, 32], F32)
            nc.vector.tensor_reduce(
                colB[:, :],
                tb1.rearrange("j (P q) -> j q P", P=4),
                axis=X,
                op=MIN,
            )
            # clamp negatives (cancellation noise), sqrt, sum over free dim
            nc.vector.tensor_scalar_max(rowmin8[:, :], rowmin8[:, :], 0.0)
            rowd = fin_pool.tile([128, N_BLK], F32)
            rowpart = fin_pool.tile([128, 1], F32)
            i_rowsqrt = nc.scalar.activation(
                rowd[:, :], rowmin8[:, :],
                mybir.ActivationFunctionType.Sqrt,
                accum_out=rowpart[:, :],
            )
            # partition sum -> scalar
            ps_row = mm_pool.tile([128, 4 * CHUNK], F32, name="ps_row", tag="mm")
            nc.tensor.matmul(
                ps_row[0:1, 0:1], ones_sb[:, :], rowpart[:, :],
                start=True, stop=True,
            )
            rowsum_sb = fin_pool.tile([1, 1], F32)
            nc.scalar.copy(rowsum_sb[:, :], ps_row[0:1, 0:1])

            # ---- AllGather #2 over [128, 33] fp32: cols 0..31 = colB,
            # col 32 row 0 = this core's rowsum. AllGather is ~2x cheaper
            # than AllReduce; the 8-way min is done locally afterwards.
            ar2_in = dram_pool.tile([128, 33], F32)
            ag2_out = dram_pool.tile([1024, 33], F32, addr_space="Shared")
            nc.sync.dma_start(ar2_in[0:128, 0:32], colB[:, :])
            i_gdma = nc.sync.dma_start(ar2_in[0:1, 32:33], rowsum_sb[:, :])
            nc.gpsimd.collective_compute(
                "AllGather",
                mybir.AluOpType.bypass,
                replica_groups=[list(range(N_CORES))],
                ins=[ar2_in[:, :].opt()],
                outs=[ag2_out[:, :].opt()],
            )

            # ---- global finalization (identical on every core) ----
            cminA = fin_pool.tile([128, 32], BF16)
            call = fin_pool.tile([128, 8 * 33], F32)
            # pin AR#1 consumers behind the tail of the main pipeline so
            # the scheduler can't park them at the head of an engine queue
            # (which would stall the whole loop on the collective)
            i_cA = nc.sync.dma_start(cminA[:, :], ar1_out[:, :])
            add_dep_helper(i_cA.ins, i_gdma.ins, sync=False,
                           reason="AR1 consumer after loop tail")
            for j in range(N_CORES):
                engs[j % 2].dma_start(
                    call[:, 33 * j:33 * (j + 1)],
                    ag2_out[128 * j:128 * (j + 1), :],
                )

            i_clA = nc.vector.tensor_scalar_max(cminA[:, :], cminA[:, :], 0.0)
            add_dep_helper(i_clA.ins, i_gdma.ins, sync=False,
                           reason="AR1 consumer after loop tail")
            # local 8-way min over the gathered colB blocks
            cv = call.rearrange("r (j q) -> r j q", j=N_CORES)
            m1 = fin_pool.tile([128, 4 * 32], F32)
            m2 = fin_pool.tile([128, 2 * 32], F32)
            cminB = fin_pool.tile([128, 32], F32)
            m1v = m1.rearrange("r (j q) -> r j q", j=4)
            m2v = m2.rearrange("r (j q) -> r j q", j=2)
            nc.vector.tensor_tensor(
                m1v[:, :, :], cv[:, 0:4, 0:32], cv[:, 4:8, 0:32], MIN
            )
            nc.vector.tensor_tensor(
                m2v[:, :, :], m1v[:, 0:2, :], m1v[:, 2:4, :], MIN
            )
            nc.vector.tensor_tensor(
                cminB.rearrange("r (j q) -> r j q", j=1),
                m2v[:, 0:1, :],
                m2v[:, 1:2, :],
                MIN,
            )
            nc.vector.tensor_scalar_max(cminB[:, :], cminB[:, :], 0.0)
            cdA = fin_pool.tile([128, 32], F32)
            cdB = fin_pool.tile([128, 32], F32)
            colpartA = fin_pool.tile([128, 1], F32)
            colpartB = fin_pool.tile([128, 1], F32)
            i_sqA = nc.scalar.activation(
                cdA[:, :], cminA[:, :],
                mybir.ActivationFunctionType.Sqrt,
                accum_out=colpartA[:, :],
            )
            add_dep_helper(i_sqA.ins, i_rowsqrt.ins, sync=False,
                           reason="AR1 consumer after loop tail")
            nc.scalar.activation(
                cdB[:, :], cminB[:, :],
                mybir.ActivationFunctionType.Sqrt,
                accum_out=colpartB[:, :],
            )
            colpart = fin_pool.tile([128, 1], F32)
            nc.vector.tensor_tensor(
                colpart[:, :], colpartA[:, :], colpartB[:, :], ADD
            )
            ps_col = mm_pool.tile([128, 4 * CHUNK], F32, name="ps_col", tag="mm")
            nc.tensor.matmul(
                ps_col[0:1, 0:1], ones_sb[:, :], colpart[:, :],
                start=True, stop=True,
            )
            colsum_sb = fin_pool.tile([1, 1], F32)
            nc.scalar.copy(colsum_sb[:, :], ps_col[0:1, 0:1])

            # rowsums: block j's col 32, row 0 -> strided [1, 8] view
            rtot = fin_pool.tile([1, 1], F32)
            nc.vector.tensor_reduce(
                rtot[:, :],
                call.rearrange("r (j q) -> r j q", j=N_CORES)[0:1, :, 32],
                axis=X,
                op=ADD,
            )

            fin = fin_pool.tile([1, 1], F32)
            nc.vector.tensor_tensor(fin[:, :], colsum_sb[:, :], rtot[:, :], ADD)
            out_sb = fin_pool.tile([1, 1], F32)
            nc.scalar.mul(out_sb[:, :], fin[:, :], 1.0 / M)
            nc.sync.dma_start(out_d.ap(), out_sb[:, :])

    nc.compile()
    return nc


def _prep_inputs(y_pred, y_true):
    p = np.ascontiguousarray(np.asarray(y_pred, dtype=np.float32).reshape(-1, 2))
    t = np.ascontiguousarray(np.asarray(y_true, dtype=np.float32).reshape(-1, 2))
    assert p.shape == (N, 2) and t.shape == (M, 2)

    rhs4 = np.empty((4, M), dtype=np.float32)
    rhs4[0] = t[:, 0]
    rhs4[1] = t[:, 1]
    rhs4[2] = t[:, 0] * t[:, 0] + t[:, 1] * t[:, 1]
    rhs4[3] = 1.0
    rhs = np.tile(rhs4, (4, 1))

    in_maps = []
    for k in range(N_CORES):
        pk = p[k * N_LOC:(k + 1) * N_LOC]
        lhs4 = np.empty((4, N_LOC), dtype=np.float32)
        lhs4[0] = -2.0 * pk[:, 0]
        lhs4[1] = -2.0 * pk[:, 1]
        lhs4[2] = 1.0
        lhs4[3] = pk[:, 0] * pk[:, 0] + pk[:, 1] * pk[:, 1]
        lhs = np.tile(lhs4, (4, 1))
        sel = np.zeros((1, 64), dtype=np.float32)
        sel[0, k] = 1.0
        selbig = np.full((1, 64), BIG, dtype=np.float32)
        selbig[0, k] = 0.0
        in_maps.append({"lhs": lhs, "rhs": rhs, "sel": sel, "selbig": selbig})
    return in_maps


def kernel(y_pred, y_true):
    global LAST_RESULTS
    if "nc" not in _CACHE:
        _CACHE["nc"] = _build_program()
    nc = _CACHE["nc"]
    in_maps = _prep_inputs(y_pred, y_true)
    res = run_bass_kernel_spmd(
        nc,
        in_maps,
        core_ids=list(range(N_CORES)),
        trace=TRACE,
    )
    LAST_RESULTS = res
    return np.asarray(res.results[0]["out"], dtype=np.float32).reshape(())[()]
